# revision 12
# baseline (speedup 1.0000x reference)
"""GT layer (graph transformer message passing) on 8 trn2 NeuronCores.

nn_GTLayer: N=100000 nodes, E=800000 edges, D=64, H=4 heads.
Self-contained: accepts FULL unsharded inputs, returns FULL [N, D] output.

Strategy (dst-node sharded, no collectives):
  - Each core owns a contiguous range of 12544 destination nodes
    (98 buckets x 128 nodes). Host routes each edge to the core/bucket of
    its destination row, pads every bucket to a uniform tile count TB so
    the 8 cores run an identical (SPMD) instruction stream.
  - Phase 1 (on device): kv[n] = [emb[n] @ Wk | emb[n] @ Wv] table written
    to DRAM ([N,128] fp32, 512B records), q = emb_own @ Wq for own nodes.
  - Phase 2 (on device): per bucket of 128 dst nodes: indirect-DMA gather
    of kv[cols] (512B/edge), one-hot matrices GT/G built from localrow via
    ACT (Square + Relu) and PE transpose, q gathered per edge by a one-hot
    matmul, scores/exp/weighting on DVE+ACT, scatter-add via one-hot
    matmul accumulating in PSUM, per-node normalization, store.
"""

import math
import os
import numpy as np

import concourse.bass as bass
import concourse.bacc as bacc
import concourse.mybir as mybir
import concourse.tile as tile
from concourse import bass_utils
from concourse.masks import make_identity

FP32 = mybir.dt.float32
BF16 = mybir.dt.bfloat16
I32 = mybir.dt.int32

N_NODES = 100000
N_EDGES = 800000
D = 64
H = 4
DH = 16
P = 128
NCORES = 8
NB = 98                      # buckets per core
NPC = NB * P                 # nodes per core (12544); last core partial
N_PAD = NCORES * NPC         # 100352 padded node count

LAST_RESULT = None           # BassKernelResults of the most recent run


# ----------------------------------------------------------------- host side
def _preprocess(edge_index):
    """Route edges to (core, bucket) by destination row; pad buckets to a
    uniform tile count TB. Returns per-core cols/localrow arrays + TB."""
    rows = edge_index[0].astype(np.int64)
    cols = edge_index[1].astype(np.int64)

    bucket = rows >> 7                         # global 128-node bucket id
    nbuck = NCORES * NB                        # 784 (padded global buckets)
    order = np.argsort(bucket, kind="stable")
    b_sorted = bucket[order]
    counts = np.bincount(b_sorted, minlength=nbuck)
    TB = max(2, int(math.ceil(counts.max() / P)))
    S = TB * P                                 # padded edges per bucket

    # position of each sorted edge within its bucket
    starts = np.zeros(nbuck + 1, dtype=np.int64)
    np.cumsum(counts, out=starts[1:])
    pos = np.arange(len(order), dtype=np.int64) - starts[b_sorted]

    flat = b_sorted * S + pos                  # slot in padded layout
    cols_pad = np.zeros(nbuck * S, dtype=np.int32)
    lrow_pad = np.full(nbuck * S, -1.0, dtype=np.float32)
    cols_pad[flat] = cols[order].astype(np.int32)
    lrow_pad[flat] = (rows[order] & 127).astype(np.float32)

    # gather offsets iterate [partition p, tile t]; slot (p,t) must hold
    # edge (t*128+p) of the bucket -> store cols as [.., 128, TB]
    cols_g = cols_pad.reshape(nbuck, TB, P).transpose(0, 2, 1).copy()
    cols_g = cols_g.reshape(NCORES, NB, P, TB)
    lrow = lrow_pad.reshape(NCORES, NB, TB * P)
    return cols_g, lrow, TB


# --------------------------------------------------------------- device side
def _build_program(TB, debug_dump=False):
    nc = bacc.Bacc("TRN2", target_bir_lowering=False, debug=False)

    emb_t = nc.dram_tensor("emb_t", [D, N_PAD], FP32, kind="ExternalInput")
    emb_own_t = nc.dram_tensor("emb_own_t", [D, NPC], FP32,
                               kind="ExternalInput")
    w_kv = nc.dram_tensor("w_kv", [D, 2 * D], FP32, kind="ExternalInput")
    w_q = nc.dram_tensor("w_q", [D, D], FP32, kind="ExternalInput")
    cols_g = nc.dram_tensor("cols_g", [NB, P, TB], I32, kind="ExternalInput")
    lrow = nc.dram_tensor("lrow", [NB, TB * P], FP32, kind="ExternalInput")
    out_d = nc.dram_tensor("out", [NPC, D], FP32, kind="ExternalOutput")

    kv_d = nc.dram_tensor("kv_scratch", [N_PAD, 2 * D], FP32)
    q_d = nc.dram_tensor("q_scratch", [NPC, D], FP32)
    dbg = {}
    if debug_dump:
        S_ = TB * P
        for name, shape in [("d_kvg", [P, S_]), ("d_gt", [P, S_]),
                            ("d_g", [P, S_]), ("d_w", [P, TB * H]),
                            ("d_acc", [P, D]), ("d_nrm", [P, H]),
                            ("d_qb", [P, D]), ("d_kv0", [2 * P, 2 * D])]:
            dbg[name] = nc.dram_tensor(name, shape, FP32,
                                       kind="ExternalOutput")

    n_tiles_full = N_PAD // P        # 784
    GRP = 8                          # node tiles per phase-1 group

    with tile.TileContext(nc) as tc:
        # ---------------- constants
        with tc.tile_pool(name="const", bufs=1) as cpool:
            wkv_sb = cpool.tile([D, 2 * D], FP32)
            nc.sync.dma_start(out=wkv_sb[:], in_=w_kv[:, :])
            wq_sb = cpool.tile([D, D], FP32)
            nc.sync.dma_start(out=wq_sb[:], in_=w_q[:, :])
            ident = cpool.tile([P, P], FP32)
            make_identity(nc, ident[:])
            neg_iota = cpool.tile([P, 1], I32)
            nc.gpsimd.iota(neg_iota[:], pattern=[[0, 1]], base=0,
                           channel_multiplier=-1)
            neg_iota_f = cpool.tile([P, 1], FP32)
            nc.vector.tensor_copy(out=neg_iota_f[:], in_=neg_iota[:])

            # ---------------- phase 1a: kv table for all nodes
            with tc.tile_pool(name="ph1", bufs=3) as pool, \
                 tc.tile_pool(name="ph1ps", bufs=2, space="PSUM") as pps:
                for g in range(n_tiles_full // GRP):        # 98 groups
                    et = pool.tile([D, GRP * P], FP32, tag="et")
                    nc.sync.dma_start(
                        out=et[:],
                        in_=emb_t[:, g * GRP * P:(g + 1) * GRP * P])
                    kv_ps = pps.tile([P, GRP * 2 * D], FP32, tag="kvps")
                    for i in range(GRP):
                        nc.tensor.matmul(
                            out=kv_ps[:, i * 2 * D:(i + 1) * 2 * D],
                            lhsT=et[:, i * P:(i + 1) * P],
                            rhs=wkv_sb[:],
                            start=True, stop=True)
                    kv_sb = pool.tile([P, GRP * 2 * D], FP32, tag="kvsb")
                    nc.vector.tensor_copy(out=kv_sb[:], in_=kv_ps[:])
                    nc.sync.dma_start(
                        out=kv_d[g * GRP * P:(g + 1) * GRP * P, :].rearrange(
                            "(i p) d -> p i d", p=P),
                        in_=kv_sb[:].rearrange("p (i d) -> p i d", i=GRP))

            # ---------------- phase 1b: q for own nodes
            with tc.tile_pool(name="ph1b", bufs=3) as pool, \
                 tc.tile_pool(name="ph1bps", bufs=2, space="PSUM") as pps:
                done = 0
                while done < NB:
                    gw = min(GRP, NB - done)
                    et = pool.tile([D, GRP * P], FP32, tag="et")
                    nc.sync.dma_start(
                        out=et[:, :gw * P],
                        in_=emb_own_t[:, done * P:(done + gw) * P])
                    q_ps = pps.tile([P, GRP * D], FP32, tag="qps")
                    for i in range(gw):
                        nc.tensor.matmul(
                            out=q_ps[:, i * D:(i + 1) * D],
                            lhsT=et[:, i * P:(i + 1) * P],
                            rhs=wq_sb[:], start=True, stop=True)
                    q_sb = pool.tile([P, GRP * D], FP32, tag="qsb")
                    nc.vector.tensor_copy(
                        out=q_sb[:, :gw * D], in_=q_ps[:, :gw * D])
                    nc.sync.dma_start(
                        out=q_d[done * P:(done + gw) * P, :].rearrange(
                            "(i p) d -> p i d", p=P),
                        in_=q_sb[:, :gw * D].rearrange(
                            "p (i d) -> p i d", i=gw))
                    done += gw

            # barrier: phase 2 gathers read kv_d/q_d (DRAM deps not tracked)
            tc.strict_bb_all_engine_barrier()

            # ---------------- phase 2: per-bucket edge processing
            S = TB * P
            n8 = (TB + 7) // 8                   # 8-tile subgroups
            with tc.tile_pool(name="ph2", bufs=2) as pool, \
                 tc.tile_pool(name="ph2ps", bufs=2, space="PSUM") as pps, \
                 tc.tile_pool(name="ph2acc", bufs=2, space="PSUM") as apps:
                for b in range(NB):
                    colsb = pool.tile([P, TB], I32, tag="colsb")
                    nc.sync.dma_start(out=colsb[:], in_=cols_g[b, :, :])
                    lrow_b = pool.tile([1, S], FP32, tag="lrowb")
                    nc.sync.dma_start(out=lrow_b[:], in_=lrow[b:b + 1, :])
                    qb = pool.tile([P, D], FP32, tag="qb")
                    nc.sync.dma_start(
                        out=qb[:], in_=q_d[b * P:(b + 1) * P, :])

                    kvg = pool.tile([P, TB * 2 * D], FP32, tag="kvg")
                    for t in range(TB):
                        nc.gpsimd.indirect_dma_start(
                            out=kvg[:, t * 2 * D:(t + 1) * 2 * D],
                            out_offset=None,
                            in_=kv_d[:, :],
                            in_offset=bass.IndirectOffsetOnAxis(
                                ap=colsb[:, t:t + 1], axis=0))

                    # one-hot GT [n, e] = relu(1 - (localrow[e] - n)^2)
                    lrow_bc = pool.tile([P, S], FP32, tag="lrowbc")
                    nc.gpsimd.partition_broadcast(lrow_bc[:], lrow_b[:])
                    sq = pool.tile([P, S], FP32, tag="sq")
                    nc.scalar.activation(
                        out=sq[:], in_=lrow_bc[:],
                        func=mybir.ActivationFunctionType.Square,
                        bias=neg_iota_f[:, 0:1], scale=1.0)
                    gt = pool.tile([P, S], FP32, tag="gt")
                    nc.scalar.activation(
                        out=gt[:], in_=sq[:],
                        func=mybir.ActivationFunctionType.Relu,
                        bias=1.0, scale=-1.0)

                    # G = transpose(GT) per tile, staged through PSUM
                    g_sb = pool.tile([P, S], FP32, tag="gsb")
                    for t4 in range((TB + 3) // 4):
                        tw = min(4, TB - t4 * 4)
                        g_ps = pps.tile([P, 4 * P], FP32, tag="gps")
                        for j in range(tw):
                            t = t4 * 4 + j
                            nc.tensor.transpose(
                                out=g_ps[:, j * P:(j + 1) * P],
                                in_=gt[:, t * P:(t + 1) * P],
                                identity=ident[:])
                        nc.vector.tensor_copy(
                            out=g_sb[:, t4 * 4 * P:t4 * 4 * P + tw * P],
                            in_=g_ps[:, :tw * P])

                    acc_ps = apps.tile([P, D], FP32, tag="accps")
                    nrm_ps = apps.tile([P, H], FP32, tag="nrmps")
                    att = pool.tile([P, TB * H], FP32, tag="att")

                    for g8 in range(n8):
                        t0 = g8 * 8
                        tw = min(8, TB - t0)
                        qe_ps = pps.tile([P, 8 * D], FP32, tag="qeps")
                        for j in range(tw):
                            t = t0 + j
                            nc.tensor.matmul(
                                out=qe_ps[:, j * D:(j + 1) * D],
                                lhsT=gt[:, t * P:(t + 1) * P],
                                rhs=qb[:], start=True, stop=True)
                        # s = q_e * k ; att = head-sum(s)
                        s_sb = pool.tile([P, 8 * D], FP32, tag="ssb")
                        kv3 = kvg[:].rearrange("p (t c) -> p t c", c=2 * D)
                        nc.vector.tensor_tensor(
                            out=s_sb[:, :tw * D],
                            in0=qe_ps[:, :tw * D],
                            in1=kv3[:, t0:t0 + tw, 0:D],
                            op=mybir.AluOpType.mult)
                        nc.vector.tensor_reduce(
                            out=att[:, t0 * H:t0 * H + tw * H],
                            in_=s_sb[:, :tw * D].rearrange(
                                "p (g d) -> p g d", d=DH),
                            axis=mybir.AxisListType.X,
                            op=mybir.AluOpType.add)

                    # clip +-10, exponentiate
                    nc.vector.tensor_scalar_min(
                        out=att[:], in0=att[:], scalar1=10.0)
                    nc.vector.tensor_scalar_max(
                        out=att[:], in0=att[:], scalar1=-10.0)
                    w_sb = pool.tile([P, TB * H], FP32, tag="wsb")
                    nc.scalar.activation(
                        out=w_sb[:], in_=att[:],
                        func=mybir.ActivationFunctionType.Exp)

                    wv = pool.tile([P, TB * D], FP32, tag="wv")
                    for g8 in range(n8):
                        t0 = g8 * 8
                        tw = min(8, TB - t0)
                        kv3 = kvg[:].rearrange("p (t c) -> p t c", c=2 * D)
                        w4 = w_sb[:, t0 * H:t0 * H + tw * H].rearrange(
                            "p (t h) -> p t h", h=H)
                        nc.vector.tensor_tensor(
                            out=wv[:, t0 * D:t0 * D + tw * D].rearrange(
                                "p (t h f) -> p t h f", h=H, f=DH),
                            in0=kv3[:, t0:t0 + tw, D:2 * D].rearrange(
                                "p t (h f) -> p t h f", h=H),
                            in1=w4.unsqueeze(3).to_broadcast((P, tw, H, DH)),
                            op=mybir.AluOpType.mult)

                    for t in range(TB):
                        nc.tensor.matmul(
                            out=acc_ps[:],
                            lhsT=g_sb[:, t * P:(t + 1) * P],
                            rhs=wv[:, t * D:(t + 1) * D],
                            start=(t == 0), stop=(t == TB - 1))
                        nc.tensor.matmul(
                            out=nrm_ps[:],
                            lhsT=g_sb[:, t * P:(t + 1) * P],
                            rhs=w_sb[:, t * H:(t + 1) * H],
                            start=(t == 0), stop=(t == TB - 1))

                    if debug_dump and b == 0:
                        nc.sync.dma_start(out=dbg["d_kvg"][:, :], in_=kvg[:])
                        nc.sync.dma_start(out=dbg["d_gt"][:, :], in_=gt[:])
                        nc.sync.dma_start(out=dbg["d_g"][:, :], in_=g_sb[:])
                        nc.sync.dma_start(out=dbg["d_w"][:, :], in_=w_sb[:])
                        nc.sync.dma_start(out=dbg["d_qb"][:, :], in_=qb[:])
                        acc_sb = pool.tile([P, D], FP32, tag="dacc")
                        nc.vector.tensor_copy(out=acc_sb[:], in_=acc_ps[:])
                        nc.sync.dma_start(out=dbg["d_acc"][:, :],
                                          in_=acc_sb[:])
                        nrm_sb = pool.tile([P, H], FP32, tag="dnrm")
                        nc.vector.tensor_copy(out=nrm_sb[:], in_=nrm_ps[:])
                        nc.sync.dma_start(out=dbg["d_nrm"][:, :],
                                          in_=nrm_sb[:])
                        kv0_sb = pool.tile([P, 2 * 2 * D], FP32, tag="dkv0")
                        nc.sync.dma_start(
                            out=kv0_sb[:].rearrange("p (i d) -> p i d", i=2),
                            in_=kv_d[0:2 * P, :].rearrange(
                                "(i p) d -> p i d", p=P))
                        nc.sync.dma_start(
                            out=dbg["d_kv0"][:, :].rearrange(
                                "(i p) d -> p i d", p=P),
                            in_=kv0_sb[:].rearrange("p (i d) -> p i d", i=2))

                    # normalize: out = acc / (norm + 1e-8)
                    rec = pool.tile([P, H], FP32, tag="rec")
                    nc.vector.tensor_scalar_add(
                        out=rec[:], in0=nrm_ps[:], scalar1=1e-8)
                    nc.vector.reciprocal(out=rec[:], in_=rec[:])
                    outf = pool.tile([P, D], FP32, tag="outf")
                    nc.vector.tensor_tensor(
                        out=outf[:].rearrange("p (h f) -> p h f", h=H),
                        in0=acc_ps[:].rearrange("p (h f) -> p h f", h=H),
                        in1=rec[:].unsqueeze(2).to_broadcast((P, H, DH)),
                        op=mybir.AluOpType.mult)
                    nc.sync.dma_start(
                        out=out_d[b * P:(b + 1) * P, :], in_=outf[:])

    nc.compile()
    return nc


# ----------------------------------------------------------------- interface
def kernel(all_embeddings, Wq, Wk, Wv, edge_index):
    global LAST_RESULT
    emb = np.ascontiguousarray(np.asarray(all_embeddings, dtype=np.float32))
    Wq = np.asarray(Wq, dtype=np.float32)
    Wk = np.asarray(Wk, dtype=np.float32)
    Wv = np.asarray(Wv, dtype=np.float32)

    cols_g, lrow, TB = _preprocess(np.asarray(edge_index))

    emb_pad = np.zeros((N_PAD, D), dtype=np.float32)
    emb_pad[:N_NODES] = emb
    emb_t = np.ascontiguousarray(emb_pad.T)           # [D, N_PAD]
    w_kv = np.ascontiguousarray(np.concatenate([Wk, Wv], axis=1))

    nc = _build_program(TB)

    in_maps = []
    for c in range(NCORES):
        in_maps.append({
            "emb_t": emb_t,
            "emb_own_t": np.ascontiguousarray(
                emb_t[:, c * NPC:(c + 1) * NPC]),
            "w_kv": w_kv,
            "w_q": np.ascontiguousarray(Wq),
            "cols_g": np.ascontiguousarray(cols_g[c]),
            "lrow": np.ascontiguousarray(lrow[c]),
        })

    trace = bool(int(os.environ.get("GT_TRACE", "0")))
    res = bass_utils.run_bass_kernel_spmd(
        nc, in_maps, core_ids=list(range(NCORES)), trace=trace)
    LAST_RESULT = res

    out = np.empty((N_NODES, D), dtype=np.float32)
    for c in range(NCORES):
        lo = c * NPC
        hi = min((c + 1) * NPC, N_NODES)
        out[lo:hi] = res.results[c]["out"][:hi - lo]
    return out


# revision 29
# speedup vs baseline: 1.0304x; 1.0304x over previous
"""GT layer (graph transformer message passing) on 8 trn2 NeuronCores.

nn_GTLayer: N=100000 nodes, E=800000 edges, D=64, H=4 heads.
Self-contained: accepts FULL unsharded inputs, returns FULL [N, D] output.

Strategy (dst-node sharded, no collectives):
  - Each core owns a contiguous range of 12544 destination nodes
    (98 buckets x 128 nodes). Host routes each edge to the core/bucket of
    its destination row, pads every bucket to a uniform tile count TB so
    the 8 cores run an identical (SPMD) instruction stream.
  - Phase 1 (on device): kv[n] = [emb[n] @ Wk | emb[n] @ Wv] table written
    to DRAM ([N,128] fp32, 512B records), q = emb_own @ Wq for own nodes.
  - Phase 2 (on device): per bucket of 128 dst nodes: indirect-DMA gather
    of kv[cols] (512B/edge), one-hot matrices GT/G built from localrow via
    ACT (Square + Relu) and PE transpose, q gathered per edge by a one-hot
    matmul, scores/exp/weighting on DVE+ACT, scatter-add via one-hot
    matmul accumulating in PSUM, per-node normalization, store.
"""

import math
import os
import numpy as np

import concourse.bass as bass
import concourse.bacc as bacc
import concourse.mybir as mybir
import concourse.tile as tile
from concourse import bass_utils
from concourse.masks import make_identity

FP32 = mybir.dt.float32
BF16 = mybir.dt.bfloat16
I32 = mybir.dt.int32

N_NODES = 100000
N_EDGES = 800000
D = 64
H = 4
DH = 16
P = 128
NCORES = 8
NB = 98                      # buckets per core
NPC = NB * P                 # nodes per core (12544); last core partial
N_PAD = NCORES * NPC         # 100352 padded node count

LAST_RESULT = None           # BassKernelResults of the most recent run


# ----------------------------------------------------------------- host side
def _preprocess(edge_index):
    """Route edges to (core, bucket) by destination row; pad buckets to a
    uniform tile count TB. Returns per-core cols/localrow arrays + TB."""
    rows = edge_index[0].astype(np.int64)
    cols = edge_index[1].astype(np.int64)

    bucket = rows >> 7                         # global 128-node bucket id
    nbuck = NCORES * NB                        # 784 (padded global buckets)
    order = np.argsort(bucket, kind="stable")
    b_sorted = bucket[order]
    counts = np.bincount(b_sorted, minlength=nbuck)
    TB = max(2, int(math.ceil(counts.max() / P)))
    S = TB * P                                 # padded edges per bucket

    # position of each sorted edge within its bucket
    starts = np.zeros(nbuck + 1, dtype=np.int64)
    np.cumsum(counts, out=starts[1:])
    pos = np.arange(len(order), dtype=np.int64) - starts[b_sorted]

    flat = b_sorted * S + pos                  # slot in padded layout
    cols_pad = np.zeros(nbuck * S, dtype=np.int32)
    lrow_pad = np.full(nbuck * S, -1.0, dtype=np.float32)
    cols_pad[flat] = cols[order].astype(np.int32)
    lrow_pad[flat] = (rows[order] & 127).astype(np.float32)

    # gather offsets iterate [partition p, tile t]; slot (p,t) must hold
    # edge (t*128+p) of the bucket -> store cols as [.., 128, TB]
    cols_g = cols_pad.reshape(nbuck, TB, P).transpose(0, 2, 1).copy()
    cols_g = cols_g.reshape(NCORES, NB, P, TB)
    lrow = lrow_pad.reshape(NCORES, NB, TB * P)
    return cols_g, lrow, TB


# --------------------------------------------------------------- device side
def _build_program(TB, debug_dump=False):
    nc = bacc.Bacc("TRN2", target_bir_lowering=False, debug=False)

    F32R = mybir.dt.float32r
    emb_t = nc.dram_tensor("emb_t", [D, N_PAD], F32R, kind="ExternalInput")
    emb_own_t = nc.dram_tensor("emb_own_t", [D, NPC], F32R,
                               kind="ExternalInput")
    w_kv = nc.dram_tensor("w_kv", [D, 2 * D], F32R, kind="ExternalInput")
    w_q = nc.dram_tensor("w_q", [D, D], F32R, kind="ExternalInput")
    cols_g = nc.dram_tensor("cols_g", [NB, P, TB], I32, kind="ExternalInput")
    lrow = nc.dram_tensor("lrow", [NB, TB * P], FP32, kind="ExternalInput")
    out_d = nc.dram_tensor("out", [NPC, D], FP32, kind="ExternalOutput")

    kv_d = nc.dram_tensor("kv_scratch", [N_PAD, 2 * D], FP32)
    q_hi_d = nc.dram_tensor("q_hi_scratch", [NPC, D], BF16)
    q_lo_d = nc.dram_tensor("q_lo_scratch", [NPC, D], BF16)
    dbg = {}
    if debug_dump:
        S_ = TB * P
        for name, shape, dt_ in [("d_kvg", [P, S_], FP32),
                                 ("d_gt", [P, S_], BF16),
                                 ("d_g", [P, S_], BF16),
                                 ("d_w", [P, TB * H], FP32),
                                 ("d_acc", [P, D], FP32),
                                 ("d_nrm", [P, H], FP32)]:
            dbg[name] = nc.dram_tensor(name, shape, dt_,
                                       kind="ExternalOutput")

    n_tiles_full = N_PAD // P        # 784
    GRP = 8                          # node tiles per phase-1 group

    with tile.TileContext(nc) as tc:
        # ---------------- constants
        with tc.tile_pool(name="const", bufs=1) as cpool:
            # [Wkv | 0] padded to 256 cols so fp32r matmul runs 1 cyc/row
            wkv_sb = cpool.tile([D, 4 * D], F32R)
            nc.vector.memset(wkv_sb[:].bitcast(FP32), 0.0)
            nc.sync.dma_start(out=wkv_sb[:, :2 * D], in_=w_kv[:, :])
            wq_sb = cpool.tile([D, 4 * D], F32R)
            nc.vector.memset(wq_sb[:].bitcast(FP32), 0.0)
            nc.sync.dma_start(out=wq_sb[:, :D], in_=w_q[:, :])
            ident = cpool.tile([P, P], FP32)
            make_identity(nc, ident[:])
            ident_bf = cpool.tile([P, P], BF16)
            make_identity(nc, ident_bf[:])
            neg_iota = cpool.tile([P, 1], I32)
            nc.gpsimd.iota(neg_iota[:], pattern=[[0, 1]], base=0,
                           channel_multiplier=-1)
            neg_iota_f = cpool.tile([P, 1], FP32)
            nc.vector.tensor_copy(out=neg_iota_f[:], in_=neg_iota[:])

            # ---------------- phase 1a: kv table for all nodes (fp32r)
            with tc.tile_pool(name="ph1", bufs=3) as pool, \
                 tc.tile_pool(name="ph1ps", bufs=2, space="PSUM") as pps:
                for g in range(n_tiles_full // GRP):        # 98 groups
                    et = pool.tile([D, GRP * P], F32R, tag="et")
                    nc.sync.dma_start(
                        out=et[:],
                        in_=emb_t[:, g * GRP * P:(g + 1) * GRP * P])
                    kv_ps = pps.tile([P, GRP * 4 * D], FP32, tag="kvps")
                    for i in range(GRP):
                        nc.tensor.matmul(
                            out=kv_ps[:, i * 4 * D:(i + 1) * 4 * D],
                            lhsT=et[:, i * P:(i + 1) * P],
                            rhs=wkv_sb[:],
                            start=True, stop=True)
                    kv_sb = pool.tile([P, GRP * 2 * D], FP32, tag="kvsb")
                    nc.vector.tensor_copy(
                        out=kv_sb[:].rearrange("p (i d) -> p i d", i=GRP),
                        in_=kv_ps[:].rearrange(
                            "p (i d) -> p i d", i=GRP)[:, :, :2 * D])
                    nc.sync.dma_start(
                        out=kv_d[g * GRP * P:(g + 1) * GRP * P, :].rearrange(
                            "(i p) d -> p i d", p=P),
                        in_=kv_sb[:].rearrange("p (i d) -> p i d", i=GRP))

            # ---------------- phase 1b: q for own nodes (hi/lo bf16 split)
            with tc.tile_pool(name="ph1b", bufs=3) as pool, \
                 tc.tile_pool(name="ph1bps", bufs=2, space="PSUM") as pps:
                done = 0
                while done < NB:
                    gw = min(GRP, NB - done)
                    et = pool.tile([D, GRP * P], F32R, tag="et")
                    nc.sync.dma_start(
                        out=et[:, :gw * P],
                        in_=emb_own_t[:, done * P:(done + gw) * P])
                    q_ps = pps.tile([P, GRP * 4 * D], FP32, tag="qps")
                    for i in range(gw):
                        nc.tensor.matmul(
                            out=q_ps[:, i * 4 * D:(i + 1) * 4 * D],
                            lhsT=et[:, i * P:(i + 1) * P],
                            rhs=wq_sb[:],
                            start=True, stop=True)
                    qp4 = q_ps[:].rearrange("p (i d) -> p i d", i=GRP)
                    q_hi = pool.tile([P, GRP * D], BF16, tag="qhi")
                    nc.vector.tensor_copy(
                        out=q_hi[:, :gw * D].rearrange(
                            "p (i d) -> p i d", i=gw),
                        in_=qp4[:, :gw, :D])
                    q_lo = pool.tile([P, GRP * D], BF16, tag="qlo")
                    nc.vector.tensor_tensor(
                        out=q_lo[:, :gw * D].rearrange(
                            "p (i d) -> p i d", i=gw),
                        in0=qp4[:, :gw, :D],
                        in1=q_hi[:, :gw * D].rearrange(
                            "p (i d) -> p i d", i=gw),
                        op=mybir.AluOpType.subtract)
                    nc.sync.dma_start(
                        out=q_hi_d[done * P:(done + gw) * P, :].rearrange(
                            "(i p) d -> p i d", p=P),
                        in_=q_hi[:, :gw * D].rearrange(
                            "p (i d) -> p i d", i=gw))
                    nc.sync.dma_start(
                        out=q_lo_d[done * P:(done + gw) * P, :].rearrange(
                            "(i p) d -> p i d", p=P),
                        in_=q_lo[:, :gw * D].rearrange(
                            "p (i d) -> p i d", i=gw))
                    done += gw

            # barrier: phase 2 gathers read kv_d/q_d (DRAM deps not tracked)
            tc.strict_bb_all_engine_barrier()

            # ---------------- phase 2: per-bucket edge processing
            S = TB * P
            n8 = (TB + 7) // 8                   # 8-tile subgroups
            with tc.tile_pool(name="ph2", bufs=2) as pool, \
                 tc.tile_pool(name="ph2ps", bufs=2, space="PSUM") as pps, \
                 tc.tile_pool(name="ph2acc", bufs=2, space="PSUM") as apps:
                for b in range(NB):
                    colsb = pool.tile([P, TB], I32, tag="colsb")
                    nc.sync.dma_start(out=colsb[:], in_=cols_g[b, :, :])
                    lrow_b = pool.tile([1, S], FP32, tag="lrowb")
                    nc.sync.dma_start(out=lrow_b[:], in_=lrow[b:b + 1, :])
                    qb_hi = pool.tile([P, D], BF16, tag="qbhi")
                    nc.sync.dma_start(
                        out=qb_hi[:], in_=q_hi_d[b * P:(b + 1) * P, :])
                    qb_lo = pool.tile([P, D], BF16, tag="qblo")
                    nc.sync.dma_start(
                        out=qb_lo[:], in_=q_lo_d[b * P:(b + 1) * P, :])

                    kvg = pool.tile([P, TB * 2 * D], FP32, tag="kvg")
                    for t in range(TB):
                        nc.gpsimd.indirect_dma_start(
                            out=kvg[:, t * 2 * D:(t + 1) * 2 * D],
                            out_offset=None,
                            in_=kv_d[:, :],
                            in_offset=bass.IndirectOffsetOnAxis(
                                ap=colsb[:, t:t + 1], axis=0))

                    # one-hot GT [n, e] = relu(1 - (localrow[e] - n)^2)
                    lrow_bc = pool.tile([P, S], FP32, tag="lrowbc")
                    nc.gpsimd.partition_broadcast(lrow_bc[:], lrow_b[:])
                    sq = pool.tile([P, S], BF16, tag="sq")
                    nc.scalar.activation(
                        out=sq[:], in_=lrow_bc[:],
                        func=mybir.ActivationFunctionType.Square,
                        bias=neg_iota_f[:, 0:1], scale=1.0)
                    gt = pool.tile([P, S], BF16, tag="gt")
                    nc.scalar.activation(
                        out=gt[:], in_=sq[:],
                        func=mybir.ActivationFunctionType.Relu,
                        bias=1.0, scale=-1.0)

                    # G = transpose(GT) per tile, staged through PSUM
                    g_sb = pool.tile([P, S], BF16, tag="gsb")
                    for t4 in range((TB + 3) // 4):
                        tw = min(4, TB - t4 * 4)
                        g_ps = pps.tile([P, 4 * P], BF16, tag="gps")
                        for j in range(tw):
                            t = t4 * 4 + j
                            nc.tensor.transpose(
                                out=g_ps[:, j * P:(j + 1) * P],
                                in_=gt[:, t * P:(t + 1) * P],
                                identity=ident_bf[:])
                        nc.vector.tensor_copy(
                            out=g_sb[:, t4 * 4 * P:t4 * 4 * P + tw * P],
                            in_=g_ps[:, :tw * P])

                    acc_ps = apps.tile([P, D], FP32, tag="accps")
                    nrm_ps = apps.tile([P, H], FP32, tag="nrmps")
                    att = pool.tile([P, TB * H], FP32, tag="att")

                    for g8 in range(n8):
                        t0 = g8 * 8
                        tw = min(8, TB - t0)
                        qe_ps = pps.tile([P, 8 * D], FP32, tag="qeps")
                        for j in range(tw):
                            t = t0 + j
                            nc.tensor.matmul(
                                out=qe_ps[:, j * D:(j + 1) * D],
                                lhsT=gt[:, t * P:(t + 1) * P],
                                rhs=qb_hi[:], start=True, stop=False)
                            nc.tensor.matmul(
                                out=qe_ps[:, j * D:(j + 1) * D],
                                lhsT=gt[:, t * P:(t + 1) * P],
                                rhs=qb_lo[:], start=False, stop=True)
                        # s = q_e * k ; att = head-sum(s)
                        s_sb = pool.tile([P, 8 * D], FP32, tag="ssb")
                        kv3 = kvg[:].rearrange("p (t c) -> p t c", c=2 * D)
                        nc.vector.tensor_tensor(
                            out=s_sb[:, :tw * D],
                            in0=qe_ps[:, :tw * D],
                            in1=kv3[:, t0:t0 + tw, 0:D],
                            op=mybir.AluOpType.mult)
                        nc.vector.tensor_reduce(
                            out=att[:, t0 * H:t0 * H + tw * H],
                            in_=s_sb[:, :tw * D].rearrange(
                                "p (g d) -> p g d", d=DH),
                            axis=mybir.AxisListType.X,
                            op=mybir.AluOpType.add)

                    # clip +-10, exponentiate
                    nc.vector.tensor_scalar_min(
                        out=att[:], in0=att[:], scalar1=10.0)
                    nc.vector.tensor_scalar_max(
                        out=att[:], in0=att[:], scalar1=-10.0)
                    w_sb = pool.tile([P, TB * H], FP32, tag="wsb")
                    nc.scalar.activation(
                        out=w_sb[:], in_=att[:],
                        func=mybir.ActivationFunctionType.Exp)
                    w_bf = pool.tile([P, TB * H], BF16, tag="wbf")
                    nc.vector.tensor_copy(out=w_bf[:], in_=w_sb[:])

                    wv = pool.tile([P, TB * D], BF16, tag="wv")
                    for g8 in range(n8):
                        t0 = g8 * 8
                        tw = min(8, TB - t0)
                        kv3 = kvg[:].rearrange("p (t c) -> p t c", c=2 * D)
                        w4 = w_sb[:, t0 * H:t0 * H + tw * H].rearrange(
                            "p (t h) -> p t h", h=H)
                        nc.vector.tensor_tensor(
                            out=wv[:, t0 * D:t0 * D + tw * D].rearrange(
                                "p (t h f) -> p t h f", h=H, f=DH),
                            in0=kv3[:, t0:t0 + tw, D:2 * D].rearrange(
                                "p t (h f) -> p t h f", h=H),
                            in1=w4.unsqueeze(3).to_broadcast((P, tw, H, DH)),
                            op=mybir.AluOpType.mult)

                    for t in range(TB):
                        nc.tensor.matmul(
                            out=acc_ps[:],
                            lhsT=g_sb[:, t * P:(t + 1) * P],
                            rhs=wv[:, t * D:(t + 1) * D],
                            start=(t == 0), stop=(t == TB - 1))
                        nc.tensor.matmul(
                            out=nrm_ps[:],
                            lhsT=g_sb[:, t * P:(t + 1) * P],
                            rhs=w_bf[:, t * H:(t + 1) * H],
                            start=(t == 0), stop=(t == TB - 1))

                    if debug_dump and b == 0:
                        nc.sync.dma_start(out=dbg["d_kvg"][:, :], in_=kvg[:])
                        nc.sync.dma_start(out=dbg["d_gt"][:, :], in_=gt[:])
                        nc.sync.dma_start(out=dbg["d_g"][:, :], in_=g_sb[:])
                        nc.sync.dma_start(out=dbg["d_w"][:, :], in_=w_sb[:])
                        acc_sb = pool.tile([P, D], FP32, tag="dacc")
                        nc.vector.tensor_copy(out=acc_sb[:], in_=acc_ps[:])
                        nc.sync.dma_start(out=dbg["d_acc"][:, :],
                                          in_=acc_sb[:])
                        nrm_sb = pool.tile([P, H], FP32, tag="dnrm")
                        nc.vector.tensor_copy(out=nrm_sb[:], in_=nrm_ps[:])
                        nc.sync.dma_start(out=dbg["d_nrm"][:, :],
                                          in_=nrm_sb[:])

                    # normalize: out = acc / (norm + 1e-8)
                    rec = pool.tile([P, H], FP32, tag="rec")
                    nc.vector.tensor_scalar_add(
                        out=rec[:], in0=nrm_ps[:], scalar1=1e-8)
                    nc.vector.reciprocal(out=rec[:], in_=rec[:])
                    outf = pool.tile([P, D], FP32, tag="outf")
                    nc.vector.tensor_tensor(
                        out=outf[:].rearrange("p (h f) -> p h f", h=H),
                        in0=acc_ps[:].rearrange("p (h f) -> p h f", h=H),
                        in1=rec[:].unsqueeze(2).to_broadcast((P, H, DH)),
                        op=mybir.AluOpType.mult)
                    nc.sync.dma_start(
                        out=out_d[b * P:(b + 1) * P, :], in_=outf[:])

    nc.compile()
    return nc


# ----------------------------------------------------------------- interface
def kernel(all_embeddings, Wq, Wk, Wv, edge_index):
    global LAST_RESULT
    emb = np.ascontiguousarray(np.asarray(all_embeddings, dtype=np.float32))
    Wq = np.asarray(Wq, dtype=np.float32)
    Wk = np.asarray(Wk, dtype=np.float32)
    Wv = np.asarray(Wv, dtype=np.float32)

    cols_g, lrow, TB = _preprocess(np.asarray(edge_index))

    emb_pad = np.zeros((N_PAD, D), dtype=np.float32)
    emb_pad[:N_NODES] = emb
    emb_t = np.ascontiguousarray(emb_pad.T)           # [D, N_PAD]
    w_kv = np.ascontiguousarray(np.concatenate([Wk, Wv], axis=1))

    nc = _build_program(TB)

    in_maps = []
    for c in range(NCORES):
        in_maps.append({
            "emb_t": emb_t,
            "emb_own_t": np.ascontiguousarray(
                emb_t[:, c * NPC:(c + 1) * NPC]),
            "w_kv": w_kv,
            "w_q": np.ascontiguousarray(Wq),
            "cols_g": np.ascontiguousarray(cols_g[c]),
            "lrow": np.ascontiguousarray(lrow[c]),
        })

    trace = bool(int(os.environ.get("GT_TRACE", "0")))
    res = bass_utils.run_bass_kernel_spmd(
        nc, in_maps, core_ids=list(range(NCORES)), trace=trace)
    LAST_RESULT = res

    out = np.empty((N_NODES, D), dtype=np.float32)
    for c in range(NCORES):
        lo = c * NPC
        hi = min((c + 1) * NPC, N_NODES)
        out[lo:hi] = res.results[c]["out"][:hi - lo]
    return out


# revision 38
# speedup vs baseline: 1.0321x; 1.0017x over previous
"""GT layer (graph transformer message passing) on 8 trn2 NeuronCores.

nn_GTLayer: N=100000 nodes, E=800000 edges, D=64, H=4 heads.
Self-contained: accepts FULL unsharded inputs, returns FULL [N, D] output.

Strategy (dst-node sharded, no collectives):
  - Each core owns a contiguous range of 12544 destination nodes
    (98 buckets x 128 nodes). Host routes each edge to the core/bucket of
    its destination row, pads every bucket to a uniform tile count TB so
    the 8 cores run an identical (SPMD) instruction stream.
  - Phase 1 (on device): kv[n] = [emb[n] @ Wk | emb[n] @ Wv] table written
    to DRAM ([N,128] fp32, 512B records), q = emb_own @ Wq for own nodes.
  - Phase 2 (on device): per bucket of 128 dst nodes: indirect-DMA gather
    of kv[cols] (512B/edge), one-hot matrices GT/G built from localrow via
    ACT (Square + Relu) and PE transpose, q gathered per edge by a one-hot
    matmul, scores/exp/weighting on DVE+ACT, scatter-add via one-hot
    matmul accumulating in PSUM, per-node normalization, store.
"""

import math
import os
import numpy as np

import concourse.bass as bass
import concourse.bacc as bacc
import concourse.mybir as mybir
import concourse.tile as tile
from concourse import bass_utils
from concourse.masks import make_identity

FP32 = mybir.dt.float32
BF16 = mybir.dt.bfloat16
I32 = mybir.dt.int32

N_NODES = 100000
N_EDGES = 800000
D = 64
H = 4
DH = 16
P = 128
NCORES = 8
NB = 98                      # buckets per core
NPC = NB * P                 # nodes per core (12544); last core partial
N_PAD = NCORES * NPC         # 100352 padded node count

LAST_RESULT = None           # BassKernelResults of the most recent run


# ----------------------------------------------------------------- host side
def _preprocess(edge_index):
    """Route edges to (core, bucket) by destination row; pad buckets to a
    uniform tile count TB. Returns per-core cols/localrow arrays + TB."""
    rows = edge_index[0].astype(np.int64)
    cols = edge_index[1].astype(np.int64)

    bucket = rows >> 7                         # global 128-node bucket id
    nbuck = NCORES * NB                        # 784 (padded global buckets)
    order = np.argsort(bucket, kind="stable")
    b_sorted = bucket[order]
    counts = np.bincount(b_sorted, minlength=nbuck)
    TB = max(2, int(math.ceil(counts.max() / P)))
    S = TB * P                                 # padded edges per bucket

    # position of each sorted edge within its bucket
    starts = np.zeros(nbuck + 1, dtype=np.int64)
    np.cumsum(counts, out=starts[1:])
    pos = np.arange(len(order), dtype=np.int64) - starts[b_sorted]

    flat = b_sorted * S + pos                  # slot in padded layout
    cols_pad = np.zeros(nbuck * S, dtype=np.int32)
    lrow_pad = np.full(nbuck * S, -1.0, dtype=np.float32)
    cols_pad[flat] = cols[order].astype(np.int32)
    lrow_pad[flat] = (rows[order] & 127).astype(np.float32)

    # gather offsets iterate [partition p, tile t]; slot (p,t) must hold
    # edge (t*128+p) of the bucket -> store cols as [.., 128, TB]
    cols_g = cols_pad.reshape(nbuck, TB, P).transpose(0, 2, 1).copy()
    cols_g = cols_g.reshape(NCORES, NB, P, TB)
    lrow = lrow_pad.reshape(NCORES, NB, TB * P)
    return cols_g, lrow, TB


# --------------------------------------------------------------- device side
def _build_program(TB, debug_dump=False):
    nc = bacc.Bacc("TRN2", target_bir_lowering=False, debug=False)

    F32R = mybir.dt.float32r
    emb_t = nc.dram_tensor("emb_t", [D, N_PAD], F32R, kind="ExternalInput")
    emb_own_t = nc.dram_tensor("emb_own_t", [D, NPC], F32R,
                               kind="ExternalInput")
    w_kv = nc.dram_tensor("w_kv", [D, 2 * D], F32R, kind="ExternalInput")
    w_q = nc.dram_tensor("w_q", [D, D], F32R, kind="ExternalInput")
    cols_g = nc.dram_tensor("cols_g", [NB, P, TB], I32, kind="ExternalInput")
    lrow = nc.dram_tensor("lrow", [NB, TB * P], FP32, kind="ExternalInput")
    out_d = nc.dram_tensor("out", [NPC, D], FP32, kind="ExternalOutput")

    kv_d = nc.dram_tensor("kv_scratch", [N_PAD, 2 * D], FP32)
    q_hi_d = nc.dram_tensor("q_hi_scratch", [NPC, D], BF16)
    q_lo_d = nc.dram_tensor("q_lo_scratch", [NPC, D], BF16)
    dbg = {}
    if debug_dump:
        S_ = TB * P
        for name, shape, dt_ in [("d_kvg", [P, S_], FP32),
                                 ("d_gt", [P, S_], BF16),
                                 ("d_g", [P, S_], BF16),
                                 ("d_w", [P, TB * H], FP32),
                                 ("d_acc", [P, D], FP32),
                                 ("d_nrm", [P, H], FP32)]:
            dbg[name] = nc.dram_tensor(name, shape, dt_,
                                       kind="ExternalOutput")

    n_tiles_full = N_PAD // P        # 784
    GRP = 8                          # node tiles per phase-1 group

    with tile.TileContext(nc) as tc:
        # ---------------- constants
        with tc.tile_pool(name="const", bufs=1) as cpool:
            # [Wkv | 0] padded to 256 cols so fp32r matmul runs 1 cyc/row
            wkv_sb = cpool.tile([D, 4 * D], F32R)
            nc.vector.memset(wkv_sb[:].bitcast(FP32), 0.0)
            nc.sync.dma_start(out=wkv_sb[:, :2 * D], in_=w_kv[:, :])
            wq_sb = cpool.tile([D, 4 * D], F32R)
            nc.vector.memset(wq_sb[:].bitcast(FP32), 0.0)
            nc.sync.dma_start(out=wq_sb[:, :D], in_=w_q[:, :])
            ident = cpool.tile([P, P], FP32)
            make_identity(nc, ident[:])
            ident_bf = cpool.tile([P, P], BF16)
            make_identity(nc, ident_bf[:])
            neg_iota = cpool.tile([P, 1], I32)
            nc.gpsimd.iota(neg_iota[:], pattern=[[0, 1]], base=0,
                           channel_multiplier=-1)
            neg_iota_f = cpool.tile([P, 1], FP32)
            nc.vector.tensor_copy(out=neg_iota_f[:], in_=neg_iota[:])

            # ---------------- phase 1a: kv table for all nodes (fp32r)
            with tc.tile_pool(name="ph1", bufs=3) as pool, \
                 tc.tile_pool(name="ph1ps", bufs=2, space="PSUM") as pps:
                for g in range(n_tiles_full // GRP):        # 98 groups
                    et = pool.tile([D, GRP * P], F32R, tag="et")
                    nc.sync.dma_start(
                        out=et[:],
                        in_=emb_t[:, g * GRP * P:(g + 1) * GRP * P])
                    kv_ps = pps.tile([P, GRP * 4 * D], FP32, tag="kvps")
                    for i in range(GRP):
                        nc.tensor.matmul(
                            out=kv_ps[:, i * 4 * D:(i + 1) * 4 * D],
                            lhsT=et[:, i * P:(i + 1) * P],
                            rhs=wkv_sb[:],
                            start=True, stop=True)
                    kv_sb = pool.tile([P, GRP * 2 * D], FP32, tag="kvsb")
                    nc.vector.tensor_copy(
                        out=kv_sb[:].rearrange("p (i d) -> p i d", i=GRP),
                        in_=kv_ps[:].rearrange(
                            "p (i d) -> p i d", i=GRP)[:, :, :2 * D])
                    nc.sync.dma_start(
                        out=kv_d[g * GRP * P:(g + 1) * GRP * P, :].rearrange(
                            "(i p) d -> p i d", p=P),
                        in_=kv_sb[:].rearrange("p (i d) -> p i d", i=GRP))

            # ---------------- phase 1b: q for own nodes (hi/lo bf16 split)
            with tc.tile_pool(name="ph1b", bufs=3) as pool, \
                 tc.tile_pool(name="ph1bps", bufs=2, space="PSUM") as pps:
                done = 0
                while done < NB:
                    gw = min(GRP, NB - done)
                    et = pool.tile([D, GRP * P], F32R, tag="et")
                    nc.sync.dma_start(
                        out=et[:, :gw * P],
                        in_=emb_own_t[:, done * P:(done + gw) * P])
                    q_ps = pps.tile([P, GRP * 4 * D], FP32, tag="qps")
                    for i in range(gw):
                        nc.tensor.matmul(
                            out=q_ps[:, i * 4 * D:(i + 1) * 4 * D],
                            lhsT=et[:, i * P:(i + 1) * P],
                            rhs=wq_sb[:],
                            start=True, stop=True)
                    qp4 = q_ps[:].rearrange("p (i d) -> p i d", i=GRP)
                    q_hi = pool.tile([P, GRP * D], BF16, tag="qhi")
                    nc.vector.tensor_copy(
                        out=q_hi[:, :gw * D].rearrange(
                            "p (i d) -> p i d", i=gw),
                        in_=qp4[:, :gw, :D])
                    q_lo = pool.tile([P, GRP * D], BF16, tag="qlo")
                    nc.vector.tensor_tensor(
                        out=q_lo[:, :gw * D].rearrange(
                            "p (i d) -> p i d", i=gw),
                        in0=qp4[:, :gw, :D],
                        in1=q_hi[:, :gw * D].rearrange(
                            "p (i d) -> p i d", i=gw),
                        op=mybir.AluOpType.subtract)
                    nc.sync.dma_start(
                        out=q_hi_d[done * P:(done + gw) * P, :].rearrange(
                            "(i p) d -> p i d", p=P),
                        in_=q_hi[:, :gw * D].rearrange(
                            "p (i d) -> p i d", i=gw))
                    nc.sync.dma_start(
                        out=q_lo_d[done * P:(done + gw) * P, :].rearrange(
                            "(i p) d -> p i d", p=P),
                        in_=q_lo[:, :gw * D].rearrange(
                            "p (i d) -> p i d", i=gw))
                    done += gw

            # barrier: phase 2 gathers read kv_d/q_d (DRAM deps not tracked)
            tc.strict_bb_all_engine_barrier()

            # ---------------- phase 2: per-bucket edge processing
            S = TB * P
            n8 = (TB + 7) // 8                   # 8-tile subgroups
            with tc.tile_pool(name="ph2", bufs=2) as pool, \
                 tc.tile_pool(name="ph2ps", bufs=2, space="PSUM") as pps, \
                 tc.tile_pool(name="ph2acc", bufs=2, space="PSUM") as apps:
                for b in range(NB):
                    colsb = pool.tile([P, TB], I32, tag="colsb")
                    nc.sync.dma_start(out=colsb[:], in_=cols_g[b, :, :])
                    lrow_b = pool.tile([1, S], FP32, tag="lrowb")
                    nc.sync.dma_start(out=lrow_b[:], in_=lrow[b:b + 1, :])
                    qb_hi = pool.tile([P, D], BF16, tag="qbhi")
                    nc.sync.dma_start(
                        out=qb_hi[:], in_=q_hi_d[b * P:(b + 1) * P, :])
                    qb_lo = pool.tile([P, D], BF16, tag="qblo")
                    nc.sync.dma_start(
                        out=qb_lo[:], in_=q_lo_d[b * P:(b + 1) * P, :])

                    kvg = pool.tile([P, TB * 2 * D], FP32, tag="kvg")
                    for t in range(TB):
                        nc.gpsimd.indirect_dma_start(
                            out=kvg[:, t * 2 * D:(t + 1) * 2 * D],
                            out_offset=None,
                            in_=kv_d[:, :],
                            in_offset=bass.IndirectOffsetOnAxis(
                                ap=colsb[:, t:t + 1], axis=0))

                    # one-hot GT [n, e] = relu(1 - (localrow[e] - n)^2)
                    lrow_bc = pool.tile([P, S], FP32, tag="lrowbc")
                    nc.gpsimd.partition_broadcast(lrow_bc[:], lrow_b[:])
                    sq = pool.tile([P, S], BF16, tag="sq")
                    nc.scalar.activation(
                        out=sq[:], in_=lrow_bc[:],
                        func=mybir.ActivationFunctionType.Square,
                        bias=neg_iota_f[:, 0:1], scale=1.0)
                    gt = pool.tile([P, S], BF16, tag="gt")
                    nc.scalar.activation(
                        out=gt[:], in_=sq[:],
                        func=mybir.ActivationFunctionType.Relu,
                        bias=1.0, scale=-1.0)

                    # G = transpose(GT) per tile, staged through PSUM
                    g_sb = pool.tile([P, S], BF16, tag="gsb")
                    for t4 in range((TB + 3) // 4):
                        tw = min(4, TB - t4 * 4)
                        g_ps = pps.tile([P, 4 * P], BF16, tag="gps")
                        for j in range(tw):
                            t = t4 * 4 + j
                            nc.tensor.transpose(
                                out=g_ps[:, j * P:(j + 1) * P],
                                in_=gt[:, t * P:(t + 1) * P],
                                identity=ident_bf[:])
                        nc.vector.tensor_copy(
                            out=g_sb[:, t4 * 4 * P:t4 * 4 * P + tw * P],
                            in_=g_ps[:, :tw * P])

                    acc_ps = apps.tile([P, D + H], FP32, tag="accps")
                    att = pool.tile([P, TB * H], FP32, tag="att")

                    for g8 in range(n8):
                        t0 = g8 * 8
                        tw = min(8, TB - t0)
                        qe_ps = pps.tile([P, 8 * D], FP32, tag="qeps")
                        for j in range(tw):
                            t = t0 + j
                            nc.tensor.matmul(
                                out=qe_ps[:, j * D:(j + 1) * D],
                                lhsT=gt[:, t * P:(t + 1) * P],
                                rhs=qb_hi[:], start=True, stop=False)
                            nc.tensor.matmul(
                                out=qe_ps[:, j * D:(j + 1) * D],
                                lhsT=gt[:, t * P:(t + 1) * P],
                                rhs=qb_lo[:], start=False, stop=True)
                        # s = q_e * k ; att = head-sum(s)
                        s_sb = pool.tile([P, 8 * D], FP32, tag="ssb")
                        kv3 = kvg[:].rearrange("p (t c) -> p t c", c=2 * D)
                        nc.vector.tensor_tensor(
                            out=s_sb[:, :tw * D],
                            in0=qe_ps[:, :tw * D],
                            in1=kv3[:, t0:t0 + tw, 0:D],
                            op=mybir.AluOpType.mult)
                        nc.vector.tensor_reduce(
                            out=att[:, t0 * H:t0 * H + tw * H],
                            in_=s_sb[:, :tw * D].rearrange(
                                "p (g d) -> p g d", d=DH),
                            axis=mybir.AxisListType.X,
                            op=mybir.AluOpType.add)

                    # clip +-10, exponentiate
                    nc.vector.tensor_scalar_min(
                        out=att[:], in0=att[:], scalar1=10.0)
                    nc.vector.tensor_scalar_max(
                        out=att[:], in0=att[:], scalar1=-10.0)
                    w_sb = pool.tile([P, TB * H], FP32, tag="wsb")
                    nc.scalar.activation(
                        out=w_sb[:], in_=att[:],
                        func=mybir.ActivationFunctionType.Exp)

                    wv = pool.tile([P, TB * (D + H)], BF16, tag="wv")
                    wv3 = wv[:].rearrange("p (t c) -> p t c", c=D + H)
                    for g8 in range(n8):
                        t0 = g8 * 8
                        tw = min(8, TB - t0)
                        kv3 = kvg[:].rearrange("p (t c) -> p t c", c=2 * D)
                        w4 = w_sb[:, t0 * H:t0 * H + tw * H].rearrange(
                            "p (t h) -> p t h", h=H)
                        nc.vector.tensor_tensor(
                            out=wv3[:, t0:t0 + tw, :D].rearrange(
                                "p t (h f) -> p t h f", h=H),
                            in0=kv3[:, t0:t0 + tw, D:2 * D].rearrange(
                                "p t (h f) -> p t h f", h=H),
                            in1=w4.unsqueeze(3).to_broadcast((P, tw, H, DH)),
                            op=mybir.AluOpType.mult)
                    nc.vector.tensor_copy(
                        out=wv3[:, :, D:],
                        in_=w_sb[:].rearrange("p (t h) -> p t h", h=H))

                    for t in range(TB):
                        nc.tensor.matmul(
                            out=acc_ps[:],
                            lhsT=g_sb[:, t * P:(t + 1) * P],
                            rhs=wv[:, t * (D + H):(t + 1) * (D + H)],
                            start=(t == 0), stop=(t == TB - 1))

                    if debug_dump and b == 0:
                        nc.sync.dma_start(out=dbg["d_kvg"][:, :], in_=kvg[:])
                        nc.sync.dma_start(out=dbg["d_gt"][:, :], in_=gt[:])
                        nc.sync.dma_start(out=dbg["d_g"][:, :], in_=g_sb[:])
                        nc.sync.dma_start(out=dbg["d_w"][:, :], in_=w_sb[:])

                    # normalize: out = acc / (norm + 1e-8)
                    rec = pool.tile([P, H], FP32, tag="rec")
                    nc.vector.tensor_scalar_add(
                        out=rec[:], in0=acc_ps[:, D:], scalar1=1e-8)
                    nc.vector.reciprocal(out=rec[:], in_=rec[:])
                    outf = pool.tile([P, D], FP32, tag="outf")
                    nc.vector.tensor_tensor(
                        out=outf[:].rearrange("p (h f) -> p h f", h=H),
                        in0=acc_ps[:, :D].rearrange("p (h f) -> p h f", h=H),
                        in1=rec[:].unsqueeze(2).to_broadcast((P, H, DH)),
                        op=mybir.AluOpType.mult)
                    nc.sync.dma_start(
                        out=out_d[b * P:(b + 1) * P, :], in_=outf[:])

    nc.compile()
    return nc


# ----------------------------------------------------------------- interface
def kernel(all_embeddings, Wq, Wk, Wv, edge_index):
    global LAST_RESULT
    emb = np.ascontiguousarray(np.asarray(all_embeddings, dtype=np.float32))
    Wq = np.asarray(Wq, dtype=np.float32)
    Wk = np.asarray(Wk, dtype=np.float32)
    Wv = np.asarray(Wv, dtype=np.float32)

    cols_g, lrow, TB = _preprocess(np.asarray(edge_index))

    emb_pad = np.zeros((N_PAD, D), dtype=np.float32)
    emb_pad[:N_NODES] = emb
    emb_t = np.ascontiguousarray(emb_pad.T)           # [D, N_PAD]
    w_kv = np.ascontiguousarray(np.concatenate([Wk, Wv], axis=1))

    nc = _build_program(TB)

    in_maps = []
    for c in range(NCORES):
        in_maps.append({
            "emb_t": emb_t,
            "emb_own_t": np.ascontiguousarray(
                emb_t[:, c * NPC:(c + 1) * NPC]),
            "w_kv": w_kv,
            "w_q": np.ascontiguousarray(Wq),
            "cols_g": np.ascontiguousarray(cols_g[c]),
            "lrow": np.ascontiguousarray(lrow[c]),
        })

    trace = bool(int(os.environ.get("GT_TRACE", "0")))
    res = bass_utils.run_bass_kernel_spmd(
        nc, in_maps, core_ids=list(range(NCORES)), trace=trace)
    LAST_RESULT = res

    out = np.empty((N_NODES, D), dtype=np.float32)
    for c in range(NCORES):
        lo = c * NPC
        hi = min((c + 1) * NPC, N_NODES)
        out[lo:hi] = res.results[c]["out"][:hi - lo]
    return out


# revision 40
# speedup vs baseline: 1.1585x; 1.1224x over previous
"""GT layer (graph transformer message passing) on 8 trn2 NeuronCores.

nn_GTLayer: N=100000 nodes, E=800000 edges, D=64, H=4 heads.
Self-contained: accepts FULL unsharded inputs, returns FULL [N, D] output.

Strategy (dst-node sharded, no collectives):
  - Each core owns a contiguous range of 12544 destination nodes
    (98 buckets x 128 nodes). Host routes each edge to the core/bucket of
    its destination row, pads every bucket to a uniform tile count TB so
    the 8 cores run an identical (SPMD) instruction stream.
  - Phase 1 (on device): kv[n] = [emb[n] @ Wk | emb[n] @ Wv] table written
    to DRAM ([N,128] fp32, 512B records), q = emb_own @ Wq for own nodes.
  - Phase 2 (on device): per bucket of 128 dst nodes: indirect-DMA gather
    of kv[cols] (512B/edge), one-hot matrices GT/G built from localrow via
    ACT (Square + Relu) and PE transpose, q gathered per edge by a one-hot
    matmul, scores/exp/weighting on DVE+ACT, scatter-add via one-hot
    matmul accumulating in PSUM, per-node normalization, store.
"""

import math
import os
import numpy as np

import concourse.bass as bass
import concourse.bacc as bacc
import concourse.mybir as mybir
import concourse.tile as tile
from concourse import bass_utils
from concourse.masks import make_identity

FP32 = mybir.dt.float32
BF16 = mybir.dt.bfloat16
I32 = mybir.dt.int32

N_NODES = 100000
N_EDGES = 800000
D = 64
H = 4
DH = 16
P = 128
NCORES = 8
NB = 98                      # buckets per core
NPC = NB * P                 # nodes per core (12544); last core partial
N_PAD = NCORES * NPC         # 100352 padded node count

LAST_RESULT = None           # BassKernelResults of the most recent run


# ----------------------------------------------------------------- host side
def _preprocess(edge_index):
    """Route edges to (core, bucket) by destination row; pad buckets to a
    uniform tile count TB. Returns per-core cols/localrow arrays + TB."""
    rows = edge_index[0].astype(np.int64)
    cols = edge_index[1].astype(np.int64)

    bucket = rows >> 7                         # global 128-node bucket id
    nbuck = NCORES * NB                        # 784 (padded global buckets)
    order = np.argsort(bucket, kind="stable")
    b_sorted = bucket[order]
    counts = np.bincount(b_sorted, minlength=nbuck)
    TB = max(2, int(math.ceil(counts.max() / P)))
    S = TB * P                                 # padded edges per bucket

    # position of each sorted edge within its bucket
    starts = np.zeros(nbuck + 1, dtype=np.int64)
    np.cumsum(counts, out=starts[1:])
    pos = np.arange(len(order), dtype=np.int64) - starts[b_sorted]

    flat = b_sorted * S + pos                  # slot in padded layout
    cols_pad = np.zeros(nbuck * S, dtype=np.int32)
    lrow_pad = np.full(nbuck * S, -1.0, dtype=np.float32)
    cols_pad[flat] = cols[order].astype(np.int32)
    lrow_pad[flat] = (rows[order] & 127).astype(np.float32)

    # gather offsets iterate [partition p, tile t]; slot (p,t) must hold
    # edge (t*128+p) of the bucket -> store cols as [.., 128, TB]
    cols_g = cols_pad.reshape(nbuck, TB, P).transpose(0, 2, 1).copy()
    cols_g = cols_g.reshape(NCORES, NB, P, TB)
    import ml_dtypes
    lr = lrow_pad.reshape(NCORES, NB, TB * P)
    lrow2 = np.empty((NCORES, NB, 2, TB * P), dtype=ml_dtypes.bfloat16)
    lrow2[:, :, 0, :] = lr.astype(ml_dtypes.bfloat16)
    lrow2[:, :, 1, :] = ml_dtypes.bfloat16(-1.0)
    return cols_g, lrow2, TB


# --------------------------------------------------------------- device side
def _build_program(TB, debug_dump=False):
    nc = bacc.Bacc("TRN2", target_bir_lowering=False, debug=False)

    F32R = mybir.dt.float32r
    emb_t = nc.dram_tensor("emb_t", [D, N_PAD], F32R, kind="ExternalInput")
    emb_own_t = nc.dram_tensor("emb_own_t", [D, NPC], F32R,
                               kind="ExternalInput")
    w_kv = nc.dram_tensor("w_kv", [D, 2 * D], F32R, kind="ExternalInput")
    w_q = nc.dram_tensor("w_q", [D, D], F32R, kind="ExternalInput")
    cols_g = nc.dram_tensor("cols_g", [NB, P, TB], I32, kind="ExternalInput")
    lrow2 = nc.dram_tensor("lrow2", [NB, 2, TB * P], BF16,
                           kind="ExternalInput")
    out_d = nc.dram_tensor("out", [NPC, D], FP32, kind="ExternalOutput")

    kv_d = nc.dram_tensor("kv_scratch", [N_PAD, 2 * D], FP32)
    q_hi_d = nc.dram_tensor("q_hi_scratch", [NPC, D], BF16)
    q_lo_d = nc.dram_tensor("q_lo_scratch", [NPC, D], BF16)
    dbg = {}
    if debug_dump:
        S_ = TB * P
        for name, shape, dt_ in [("d_kvg", [P, S_], FP32),
                                 ("d_gt", [P, S_], BF16),
                                 ("d_g", [P, S_], BF16),
                                 ("d_w", [P, TB * H], FP32),
                                 ("d_acc", [P, D], FP32),
                                 ("d_nrm", [P, H], FP32)]:
            dbg[name] = nc.dram_tensor(name, shape, dt_,
                                       kind="ExternalOutput")

    n_tiles_full = N_PAD // P        # 784
    GRP = 8                          # node tiles per phase-1 group

    with tile.TileContext(nc) as tc:
        # ---------------- constants
        with tc.tile_pool(name="const", bufs=1) as cpool:
            # [Wkv | 0] padded to 256 cols so fp32r matmul runs 1 cyc/row
            wkv_sb = cpool.tile([D, 4 * D], F32R)
            nc.vector.memset(wkv_sb[:].bitcast(FP32), 0.0)
            nc.sync.dma_start(out=wkv_sb[:, :2 * D], in_=w_kv[:, :])
            wq_sb = cpool.tile([D, 4 * D], F32R)
            nc.vector.memset(wq_sb[:].bitcast(FP32), 0.0)
            nc.sync.dma_start(out=wq_sb[:, :D], in_=w_q[:, :])
            ident = cpool.tile([P, P], FP32)
            make_identity(nc, ident[:])
            ident_bf = cpool.tile([P, P], BF16)
            make_identity(nc, ident_bf[:])
            bc_lhsT = cpool.tile([2, P], BF16)
            iota_row = cpool.tile([1, P], I32)
            nc.gpsimd.iota(iota_row[:], pattern=[[1, P]], base=0,
                           channel_multiplier=0)
            iota_bf = cpool.tile([1, P], BF16)
            nc.vector.tensor_copy(out=iota_bf[:], in_=iota_row[:])
            ones_bf = cpool.tile([1, P], BF16)
            nc.gpsimd.memset(ones_bf[:], 1.0)
            nc.sync.dma_start(out=bc_lhsT[0:1, :], in_=ones_bf[:])
            nc.sync.dma_start(out=bc_lhsT[1:2, :], in_=iota_bf[:])
            neg_iota = cpool.tile([P, 1], I32)
            nc.gpsimd.iota(neg_iota[:], pattern=[[0, 1]], base=0,
                           channel_multiplier=-1)
            neg_iota_f = cpool.tile([P, 1], FP32)
            nc.vector.tensor_copy(out=neg_iota_f[:], in_=neg_iota[:])

            # ---------------- phase 1a: kv table for all nodes (fp32r)
            with tc.tile_pool(name="ph1", bufs=3) as pool, \
                 tc.tile_pool(name="ph1ps", bufs=2, space="PSUM") as pps:
                for g in range(n_tiles_full // GRP):        # 98 groups
                    et = pool.tile([D, GRP * P], F32R, tag="et")
                    nc.sync.dma_start(
                        out=et[:],
                        in_=emb_t[:, g * GRP * P:(g + 1) * GRP * P])
                    kv_ps = pps.tile([P, GRP * 4 * D], FP32, tag="kvps")
                    for i in range(GRP):
                        nc.tensor.matmul(
                            out=kv_ps[:, i * 4 * D:(i + 1) * 4 * D],
                            lhsT=et[:, i * P:(i + 1) * P],
                            rhs=wkv_sb[:],
                            start=True, stop=True)
                    kv_sb = pool.tile([P, GRP * 2 * D], FP32, tag="kvsb")
                    nc.vector.tensor_copy(
                        out=kv_sb[:].rearrange("p (i d) -> p i d", i=GRP),
                        in_=kv_ps[:].rearrange(
                            "p (i d) -> p i d", i=GRP)[:, :, :2 * D])
                    nc.sync.dma_start(
                        out=kv_d[g * GRP * P:(g + 1) * GRP * P, :].rearrange(
                            "(i p) d -> p i d", p=P),
                        in_=kv_sb[:].rearrange("p (i d) -> p i d", i=GRP))

            # ---------------- phase 1b: q for own nodes (hi/lo bf16 split)
            with tc.tile_pool(name="ph1b", bufs=3) as pool, \
                 tc.tile_pool(name="ph1bps", bufs=2, space="PSUM") as pps:
                done = 0
                while done < NB:
                    gw = min(GRP, NB - done)
                    et = pool.tile([D, GRP * P], F32R, tag="et")
                    nc.sync.dma_start(
                        out=et[:, :gw * P],
                        in_=emb_own_t[:, done * P:(done + gw) * P])
                    q_ps = pps.tile([P, GRP * 4 * D], FP32, tag="qps")
                    for i in range(gw):
                        nc.tensor.matmul(
                            out=q_ps[:, i * 4 * D:(i + 1) * 4 * D],
                            lhsT=et[:, i * P:(i + 1) * P],
                            rhs=wq_sb[:],
                            start=True, stop=True)
                    qp4 = q_ps[:].rearrange("p (i d) -> p i d", i=GRP)
                    q_hi = pool.tile([P, GRP * D], BF16, tag="qhi")
                    nc.vector.tensor_copy(
                        out=q_hi[:, :gw * D].rearrange(
                            "p (i d) -> p i d", i=gw),
                        in_=qp4[:, :gw, :D])
                    q_lo = pool.tile([P, GRP * D], BF16, tag="qlo")
                    nc.vector.tensor_tensor(
                        out=q_lo[:, :gw * D].rearrange(
                            "p (i d) -> p i d", i=gw),
                        in0=qp4[:, :gw, :D],
                        in1=q_hi[:, :gw * D].rearrange(
                            "p (i d) -> p i d", i=gw),
                        op=mybir.AluOpType.subtract)
                    nc.sync.dma_start(
                        out=q_hi_d[done * P:(done + gw) * P, :].rearrange(
                            "(i p) d -> p i d", p=P),
                        in_=q_hi[:, :gw * D].rearrange(
                            "p (i d) -> p i d", i=gw))
                    nc.sync.dma_start(
                        out=q_lo_d[done * P:(done + gw) * P, :].rearrange(
                            "(i p) d -> p i d", p=P),
                        in_=q_lo[:, :gw * D].rearrange(
                            "p (i d) -> p i d", i=gw))
                    done += gw

            # barrier: phase 2 gathers read kv_d/q_d (DRAM deps not tracked)
            tc.strict_bb_all_engine_barrier()

            # ---------------- phase 2: per-bucket edge processing
            S = TB * P
            n8 = (TB + 7) // 8                   # 8-tile subgroups
            with tc.tile_pool(name="ph2", bufs=3) as pool, \
                 tc.tile_pool(name="ph2ps", bufs=2, space="PSUM") as pps, \
                 tc.tile_pool(name="ph2acc", bufs=2, space="PSUM") as apps:
                for b in range(NB):
                    colsb = pool.tile([P, TB], I32, tag="colsb")
                    nc.sync.dma_start(out=colsb[:], in_=cols_g[b, :, :])
                    lrow_b = pool.tile([2, S], BF16, tag="lrowb")
                    nc.sync.dma_start(out=lrow_b[:], in_=lrow2[b, :, :])
                    qb_hi = pool.tile([P, D], BF16, tag="qbhi")
                    nc.sync.dma_start(
                        out=qb_hi[:], in_=q_hi_d[b * P:(b + 1) * P, :])
                    qb_lo = pool.tile([P, D], BF16, tag="qblo")
                    nc.sync.dma_start(
                        out=qb_lo[:], in_=q_lo_d[b * P:(b + 1) * P, :])

                    kvg = pool.tile([P, TB * 2 * D], FP32, tag="kvg")
                    for t in range(TB):
                        nc.gpsimd.indirect_dma_start(
                            out=kvg[:, t * 2 * D:(t + 1) * 2 * D],
                            out_offset=None,
                            in_=kv_d[:, :],
                            in_offset=bass.IndirectOffsetOnAxis(
                                ap=colsb[:, t:t + 1], axis=0))

                    # one-hot GT [n, e] = relu(1 - (localrow[e] - n)^2)
                    # diff[n,e] = 1*lrow[e] + n*(-1) via K=2 matmul on PE
                    sq = pool.tile([P, S], BF16, tag="sq")
                    done_c = 0
                    while done_c < S:
                        cw = min(512, S - done_c)
                        dif = pps.tile([P, 512], FP32, tag="difps")
                        nc.tensor.matmul(
                            out=dif[:, :cw],
                            lhsT=bc_lhsT[:],
                            rhs=lrow_b[:, done_c:done_c + cw],
                            start=True, stop=True)
                        nc.scalar.activation(
                            out=sq[:, done_c:done_c + cw],
                            in_=dif[:, :cw],
                            func=mybir.ActivationFunctionType.Square,
                            bias=0.0, scale=1.0)
                        done_c += cw
                    gt = pool.tile([P, S], BF16, tag="gt")
                    nc.scalar.activation(
                        out=gt[:], in_=sq[:],
                        func=mybir.ActivationFunctionType.Relu,
                        bias=1.0, scale=-1.0)

                    # G = transpose(GT) per tile, staged through PSUM
                    g_sb = pool.tile([P, S], BF16, tag="gsb")
                    for t4 in range((TB + 3) // 4):
                        tw = min(4, TB - t4 * 4)
                        g_ps = pps.tile([P, 4 * P], BF16, tag="gps")
                        for j in range(tw):
                            t = t4 * 4 + j
                            nc.tensor.transpose(
                                out=g_ps[:, j * P:(j + 1) * P],
                                in_=gt[:, t * P:(t + 1) * P],
                                identity=ident_bf[:])
                        nc.vector.tensor_copy(
                            out=g_sb[:, t4 * 4 * P:t4 * 4 * P + tw * P],
                            in_=g_ps[:, :tw * P])

                    acc_ps = apps.tile([P, D + H], FP32, tag="accps")
                    att = pool.tile([P, TB * H], FP32, tag="att")

                    for g8 in range(n8):
                        t0 = g8 * 8
                        tw = min(8, TB - t0)
                        qe_ps = pps.tile([P, 8 * D], FP32, tag="qeps")
                        for j in range(tw):
                            t = t0 + j
                            nc.tensor.matmul(
                                out=qe_ps[:, j * D:(j + 1) * D],
                                lhsT=gt[:, t * P:(t + 1) * P],
                                rhs=qb_hi[:], start=True, stop=False)
                            nc.tensor.matmul(
                                out=qe_ps[:, j * D:(j + 1) * D],
                                lhsT=gt[:, t * P:(t + 1) * P],
                                rhs=qb_lo[:], start=False, stop=True)
                        # s = q_e * k ; att = head-sum(s)
                        s_sb = pool.tile([P, 8 * D], FP32, tag="ssb")
                        kv3 = kvg[:].rearrange("p (t c) -> p t c", c=2 * D)
                        nc.vector.tensor_tensor(
                            out=s_sb[:, :tw * D],
                            in0=qe_ps[:, :tw * D],
                            in1=kv3[:, t0:t0 + tw, 0:D],
                            op=mybir.AluOpType.mult)
                        nc.vector.tensor_reduce(
                            out=att[:, t0 * H:t0 * H + tw * H],
                            in_=s_sb[:, :tw * D].rearrange(
                                "p (g d) -> p g d", d=DH),
                            axis=mybir.AxisListType.X,
                            op=mybir.AluOpType.add)

                    # clip +-10, exponentiate
                    nc.vector.tensor_scalar_min(
                        out=att[:], in0=att[:], scalar1=10.0)
                    nc.vector.tensor_scalar_max(
                        out=att[:], in0=att[:], scalar1=-10.0)
                    w_sb = pool.tile([P, TB * H], FP32, tag="wsb")
                    nc.scalar.activation(
                        out=w_sb[:], in_=att[:],
                        func=mybir.ActivationFunctionType.Exp)

                    wv = pool.tile([P, TB * (D + H)], BF16, tag="wv")
                    wv3 = wv[:].rearrange("p (t c) -> p t c", c=D + H)
                    for g8 in range(n8):
                        t0 = g8 * 8
                        tw = min(8, TB - t0)
                        kv3 = kvg[:].rearrange("p (t c) -> p t c", c=2 * D)
                        w4 = w_sb[:, t0 * H:t0 * H + tw * H].rearrange(
                            "p (t h) -> p t h", h=H)
                        nc.vector.tensor_tensor(
                            out=wv3[:, t0:t0 + tw, :D].rearrange(
                                "p t (h f) -> p t h f", h=H),
                            in0=kv3[:, t0:t0 + tw, D:2 * D].rearrange(
                                "p t (h f) -> p t h f", h=H),
                            in1=w4.unsqueeze(3).to_broadcast((P, tw, H, DH)),
                            op=mybir.AluOpType.mult)
                    nc.vector.tensor_copy(
                        out=wv3[:, :, D:],
                        in_=w_sb[:].rearrange("p (t h) -> p t h", h=H))

                    for t in range(TB):
                        nc.tensor.matmul(
                            out=acc_ps[:],
                            lhsT=g_sb[:, t * P:(t + 1) * P],
                            rhs=wv[:, t * (D + H):(t + 1) * (D + H)],
                            start=(t == 0), stop=(t == TB - 1))

                    if debug_dump and b == 0:
                        nc.sync.dma_start(out=dbg["d_kvg"][:, :], in_=kvg[:])
                        nc.sync.dma_start(out=dbg["d_gt"][:, :], in_=gt[:])
                        nc.sync.dma_start(out=dbg["d_g"][:, :], in_=g_sb[:])
                        nc.sync.dma_start(out=dbg["d_w"][:, :], in_=w_sb[:])

                    # normalize: out = acc / (norm + 1e-8)
                    rec = pool.tile([P, H], FP32, tag="rec")
                    nc.vector.tensor_scalar_add(
                        out=rec[:], in0=acc_ps[:, D:], scalar1=1e-8)
                    nc.vector.reciprocal(out=rec[:], in_=rec[:])
                    outf = pool.tile([P, D], FP32, tag="outf")
                    nc.vector.tensor_tensor(
                        out=outf[:].rearrange("p (h f) -> p h f", h=H),
                        in0=acc_ps[:, :D].rearrange("p (h f) -> p h f", h=H),
                        in1=rec[:].unsqueeze(2).to_broadcast((P, H, DH)),
                        op=mybir.AluOpType.mult)
                    nc.sync.dma_start(
                        out=out_d[b * P:(b + 1) * P, :], in_=outf[:])

    nc.compile()
    return nc


# ----------------------------------------------------------------- interface
def kernel(all_embeddings, Wq, Wk, Wv, edge_index):
    global LAST_RESULT
    emb = np.ascontiguousarray(np.asarray(all_embeddings, dtype=np.float32))
    Wq = np.asarray(Wq, dtype=np.float32)
    Wk = np.asarray(Wk, dtype=np.float32)
    Wv = np.asarray(Wv, dtype=np.float32)

    cols_g, lrow2, TB = _preprocess(np.asarray(edge_index))

    emb_pad = np.zeros((N_PAD, D), dtype=np.float32)
    emb_pad[:N_NODES] = emb
    emb_t = np.ascontiguousarray(emb_pad.T)           # [D, N_PAD]
    w_kv = np.ascontiguousarray(np.concatenate([Wk, Wv], axis=1))

    nc = _build_program(TB)

    in_maps = []
    for c in range(NCORES):
        in_maps.append({
            "emb_t": emb_t,
            "emb_own_t": np.ascontiguousarray(
                emb_t[:, c * NPC:(c + 1) * NPC]),
            "w_kv": w_kv,
            "w_q": np.ascontiguousarray(Wq),
            "cols_g": np.ascontiguousarray(cols_g[c]),
            "lrow2": np.ascontiguousarray(lrow2[c]),
        })

    trace = bool(int(os.environ.get("GT_TRACE", "0")))
    res = bass_utils.run_bass_kernel_spmd(
        nc, in_maps, core_ids=list(range(NCORES)), trace=trace)
    LAST_RESULT = res

    out = np.empty((N_NODES, D), dtype=np.float32)
    for c in range(NCORES):
        lo = c * NPC
        hi = min((c + 1) * NPC, N_NODES)
        out[lo:hi] = res.results[c]["out"][:hi - lo]
    return out


# revision 41
# speedup vs baseline: 1.1594x; 1.0008x over previous
"""GT layer (graph transformer message passing) on 8 trn2 NeuronCores.

nn_GTLayer: N=100000 nodes, E=800000 edges, D=64, H=4 heads.
Self-contained: accepts FULL unsharded inputs, returns FULL [N, D] output.

Strategy (dst-node sharded, no collectives):
  - Each core owns a contiguous range of 12544 destination nodes
    (98 buckets x 128 nodes). Host routes each edge to the core/bucket of
    its destination row, pads every bucket to a uniform tile count TB so
    the 8 cores run an identical (SPMD) instruction stream.
  - Phase 1 (on device): kv[n] = [emb[n] @ Wk | emb[n] @ Wv] table written
    to DRAM ([N,128] fp32, 512B records), q = emb_own @ Wq for own nodes.
  - Phase 2 (on device): per bucket of 128 dst nodes: indirect-DMA gather
    of kv[cols] (512B/edge), one-hot matrices GT/G built from localrow via
    ACT (Square + Relu) and PE transpose, q gathered per edge by a one-hot
    matmul, scores/exp/weighting on DVE+ACT, scatter-add via one-hot
    matmul accumulating in PSUM, per-node normalization, store.
"""

import math
import os
import numpy as np

import concourse.bass as bass
import concourse.bacc as bacc
import concourse.mybir as mybir
import concourse.tile as tile
from concourse import bass_utils
from concourse.masks import make_identity

FP32 = mybir.dt.float32
BF16 = mybir.dt.bfloat16
I32 = mybir.dt.int32

N_NODES = 100000
N_EDGES = 800000
D = 64
H = 4
DH = 16
P = 128
NCORES = 8
NB = 98                      # buckets per core
NPC = NB * P                 # nodes per core (12544); last core partial
N_PAD = NCORES * NPC         # 100352 padded node count

LAST_RESULT = None           # BassKernelResults of the most recent run


# ----------------------------------------------------------------- host side
def _preprocess(edge_index):
    """Route edges to (core, bucket) by destination row; pad buckets to a
    uniform tile count TB. Returns per-core cols/localrow arrays + TB."""
    rows = edge_index[0].astype(np.int64)
    cols = edge_index[1].astype(np.int64)

    bucket = rows >> 7                         # global 128-node bucket id
    nbuck = NCORES * NB                        # 784 (padded global buckets)
    order = np.argsort(bucket, kind="stable")
    b_sorted = bucket[order]
    counts = np.bincount(b_sorted, minlength=nbuck)
    TB = max(2, int(math.ceil(counts.max() / P)))
    S = TB * P                                 # padded edges per bucket

    # position of each sorted edge within its bucket
    starts = np.zeros(nbuck + 1, dtype=np.int64)
    np.cumsum(counts, out=starts[1:])
    pos = np.arange(len(order), dtype=np.int64) - starts[b_sorted]

    flat = b_sorted * S + pos                  # slot in padded layout
    cols_pad = np.zeros(nbuck * S, dtype=np.int32)
    lrow_pad = np.full(nbuck * S, -1.0, dtype=np.float32)
    cols_pad[flat] = cols[order].astype(np.int32)
    lrow_pad[flat] = (rows[order] & 127).astype(np.float32)

    # gather offsets iterate [partition p, tile t]; slot (p,t) must hold
    # edge (t*128+p) of the bucket -> store cols as [.., 128, TB]
    cols_g = cols_pad.reshape(nbuck, TB, P).transpose(0, 2, 1).copy()
    cols_g = cols_g.reshape(NCORES, NB, P, TB)
    import ml_dtypes
    lr = lrow_pad.reshape(NCORES, NB, TB * P)
    lrow2 = np.empty((NCORES, NB, 2, TB * P), dtype=ml_dtypes.bfloat16)
    lrow2[:, :, 0, :] = lr.astype(ml_dtypes.bfloat16)
    lrow2[:, :, 1, :] = ml_dtypes.bfloat16(-1.0)
    return cols_g, lrow2, TB


# --------------------------------------------------------------- device side
def _build_program(TB, debug_dump=False):
    nc = bacc.Bacc("TRN2", target_bir_lowering=False, debug=False)

    F32R = mybir.dt.float32r
    emb_t = nc.dram_tensor("emb_t", [D, N_PAD], F32R, kind="ExternalInput")
    emb_own_t = nc.dram_tensor("emb_own_t", [D, NPC], F32R,
                               kind="ExternalInput")
    w_kv = nc.dram_tensor("w_kv", [D, 2 * D], F32R, kind="ExternalInput")
    w_q = nc.dram_tensor("w_q", [D, D], F32R, kind="ExternalInput")
    cols_g = nc.dram_tensor("cols_g", [NB, P, TB], I32, kind="ExternalInput")
    lrow2 = nc.dram_tensor("lrow2", [NB, 2, TB * P], BF16,
                           kind="ExternalInput")
    out_d = nc.dram_tensor("out", [NPC, D], FP32, kind="ExternalOutput")

    kv_d = nc.dram_tensor("kv_scratch", [N_PAD, 2 * D], FP32)
    q_hi_d = nc.dram_tensor("q_hi_scratch", [NPC, D], BF16)
    q_lo_d = nc.dram_tensor("q_lo_scratch", [NPC, D], BF16)
    dbg = {}
    if debug_dump:
        S_ = TB * P
        for name, shape, dt_ in [("d_kvg", [P, S_], FP32),
                                 ("d_gt", [P, S_], BF16),
                                 ("d_g", [P, S_], BF16),
                                 ("d_w", [P, TB * H], FP32),
                                 ("d_acc", [P, D], FP32),
                                 ("d_nrm", [P, H], FP32)]:
            dbg[name] = nc.dram_tensor(name, shape, dt_,
                                       kind="ExternalOutput")

    n_tiles_full = N_PAD // P        # 784
    GRP = 8                          # node tiles per phase-1 group

    with tile.TileContext(nc) as tc:
        # ---------------- constants
        with tc.tile_pool(name="const", bufs=1) as cpool:
            # [Wkv | 0] padded to 256 cols so fp32r matmul runs 1 cyc/row
            wkv_sb = cpool.tile([D, 4 * D], F32R)
            nc.vector.memset(wkv_sb[:].bitcast(FP32), 0.0)
            nc.sync.dma_start(out=wkv_sb[:, :2 * D], in_=w_kv[:, :])
            wq_sb = cpool.tile([D, 4 * D], F32R)
            nc.vector.memset(wq_sb[:].bitcast(FP32), 0.0)
            nc.sync.dma_start(out=wq_sb[:, :D], in_=w_q[:, :])
            ident = cpool.tile([P, P], FP32)
            make_identity(nc, ident[:])
            ident_bf = cpool.tile([P, P], BF16)
            make_identity(nc, ident_bf[:])
            bc_lhsT = cpool.tile([2, P], BF16)
            iota_row = cpool.tile([1, P], I32)
            nc.gpsimd.iota(iota_row[:], pattern=[[1, P]], base=0,
                           channel_multiplier=0)
            iota_bf = cpool.tile([1, P], BF16)
            nc.vector.tensor_copy(out=iota_bf[:], in_=iota_row[:])
            ones_bf = cpool.tile([1, P], BF16)
            nc.gpsimd.memset(ones_bf[:], 1.0)
            nc.sync.dma_start(out=bc_lhsT[0:1, :], in_=ones_bf[:])
            nc.sync.dma_start(out=bc_lhsT[1:2, :], in_=iota_bf[:])
            neg_iota = cpool.tile([P, 1], I32)
            nc.gpsimd.iota(neg_iota[:], pattern=[[0, 1]], base=0,
                           channel_multiplier=-1)
            neg_iota_f = cpool.tile([P, 1], FP32)
            nc.vector.tensor_copy(out=neg_iota_f[:], in_=neg_iota[:])

            # ---------------- phase 1a: kv table for all nodes (fp32r)
            with tc.tile_pool(name="ph1", bufs=3) as pool, \
                 tc.tile_pool(name="ph1ps", bufs=2, space="PSUM") as pps:
                for g in range(n_tiles_full // GRP):        # 98 groups
                    et = pool.tile([D, GRP * P], F32R, tag="et")
                    nc.sync.dma_start(
                        out=et[:],
                        in_=emb_t[:, g * GRP * P:(g + 1) * GRP * P])
                    kv_ps = pps.tile([P, GRP * 4 * D], FP32, tag="kvps")
                    for i in range(GRP):
                        nc.tensor.matmul(
                            out=kv_ps[:, i * 4 * D:(i + 1) * 4 * D],
                            lhsT=et[:, i * P:(i + 1) * P],
                            rhs=wkv_sb[:],
                            start=True, stop=True)
                    kv_sb = pool.tile([P, GRP * 2 * D], FP32, tag="kvsb")
                    nc.vector.tensor_copy(
                        out=kv_sb[:].rearrange("p (i d) -> p i d", i=GRP),
                        in_=kv_ps[:].rearrange(
                            "p (i d) -> p i d", i=GRP)[:, :, :2 * D])
                    nc.sync.dma_start(
                        out=kv_d[g * GRP * P:(g + 1) * GRP * P, :].rearrange(
                            "(i p) d -> p i d", p=P),
                        in_=kv_sb[:].rearrange("p (i d) -> p i d", i=GRP))

            # ---------------- phase 1b: q for own nodes (hi/lo bf16 split)
            with tc.tile_pool(name="ph1b", bufs=3) as pool, \
                 tc.tile_pool(name="ph1bps", bufs=2, space="PSUM") as pps:
                done = 0
                while done < NB:
                    gw = min(GRP, NB - done)
                    et = pool.tile([D, GRP * P], F32R, tag="et")
                    nc.sync.dma_start(
                        out=et[:, :gw * P],
                        in_=emb_own_t[:, done * P:(done + gw) * P])
                    q_ps = pps.tile([P, GRP * 4 * D], FP32, tag="qps")
                    for i in range(gw):
                        nc.tensor.matmul(
                            out=q_ps[:, i * 4 * D:(i + 1) * 4 * D],
                            lhsT=et[:, i * P:(i + 1) * P],
                            rhs=wq_sb[:],
                            start=True, stop=True)
                    qp4 = q_ps[:].rearrange("p (i d) -> p i d", i=GRP)
                    q_hi = pool.tile([P, GRP * D], BF16, tag="qhi")
                    nc.vector.tensor_copy(
                        out=q_hi[:, :gw * D].rearrange(
                            "p (i d) -> p i d", i=gw),
                        in_=qp4[:, :gw, :D])
                    q_lo = pool.tile([P, GRP * D], BF16, tag="qlo")
                    nc.vector.tensor_tensor(
                        out=q_lo[:, :gw * D].rearrange(
                            "p (i d) -> p i d", i=gw),
                        in0=qp4[:, :gw, :D],
                        in1=q_hi[:, :gw * D].rearrange(
                            "p (i d) -> p i d", i=gw),
                        op=mybir.AluOpType.subtract)
                    nc.sync.dma_start(
                        out=q_hi_d[done * P:(done + gw) * P, :].rearrange(
                            "(i p) d -> p i d", p=P),
                        in_=q_hi[:, :gw * D].rearrange(
                            "p (i d) -> p i d", i=gw))
                    nc.sync.dma_start(
                        out=q_lo_d[done * P:(done + gw) * P, :].rearrange(
                            "(i p) d -> p i d", p=P),
                        in_=q_lo[:, :gw * D].rearrange(
                            "p (i d) -> p i d", i=gw))
                    done += gw

            # barrier: phase 2 gathers read kv_d/q_d (DRAM deps not tracked)
            tc.strict_bb_all_engine_barrier()

            # ---------------- phase 2: per-bucket edge processing
            S = TB * P
            n8 = (TB + 7) // 8                   # 8-tile subgroups
            with tc.tile_pool(name="ph2", bufs=3) as pool, \
                 tc.tile_pool(name="ph2ps", bufs=2, space="PSUM") as pps, \
                 tc.tile_pool(name="ph2acc", bufs=2, space="PSUM") as apps:
                for b in range(NB):
                    colsb = pool.tile([P, TB], I32, tag="colsb")
                    nc.sync.dma_start(out=colsb[:], in_=cols_g[b, :, :])
                    lrow_b = pool.tile([2, S], BF16, tag="lrowb")
                    nc.sync.dma_start(out=lrow_b[:], in_=lrow2[b, :, :])
                    qb_hi = pool.tile([P, D], BF16, tag="qbhi")
                    nc.sync.dma_start(
                        out=qb_hi[:], in_=q_hi_d[b * P:(b + 1) * P, :])
                    qb_lo = pool.tile([P, D], BF16, tag="qblo")
                    nc.sync.dma_start(
                        out=qb_lo[:], in_=q_lo_d[b * P:(b + 1) * P, :])

                    kvg = pool.tile([P, TB * 2 * D], FP32, tag="kvg",
                                    bufs=4)
                    for t in range(TB):
                        nc.gpsimd.indirect_dma_start(
                            out=kvg[:, t * 2 * D:(t + 1) * 2 * D],
                            out_offset=None,
                            in_=kv_d[:, :],
                            in_offset=bass.IndirectOffsetOnAxis(
                                ap=colsb[:, t:t + 1], axis=0))

                    # one-hot GT [n, e] = relu(1 - (localrow[e] - n)^2)
                    # diff[n,e] = 1*lrow[e] + n*(-1) via K=2 matmul on PE
                    sq = pool.tile([P, S], BF16, tag="sq")
                    done_c = 0
                    while done_c < S:
                        cw = min(512, S - done_c)
                        dif = pps.tile([P, 512], FP32, tag="difps")
                        nc.tensor.matmul(
                            out=dif[:, :cw],
                            lhsT=bc_lhsT[:],
                            rhs=lrow_b[:, done_c:done_c + cw],
                            start=True, stop=True)
                        nc.scalar.activation(
                            out=sq[:, done_c:done_c + cw],
                            in_=dif[:, :cw],
                            func=mybir.ActivationFunctionType.Square,
                            bias=0.0, scale=1.0)
                        done_c += cw
                    gt = pool.tile([P, S], BF16, tag="gt")
                    nc.scalar.activation(
                        out=gt[:], in_=sq[:],
                        func=mybir.ActivationFunctionType.Relu,
                        bias=1.0, scale=-1.0)

                    # G = transpose(GT) per tile, staged through PSUM
                    g_sb = pool.tile([P, S], BF16, tag="gsb")
                    for t4 in range((TB + 3) // 4):
                        tw = min(4, TB - t4 * 4)
                        g_ps = pps.tile([P, 4 * P], BF16, tag="gps")
                        for j in range(tw):
                            t = t4 * 4 + j
                            nc.tensor.transpose(
                                out=g_ps[:, j * P:(j + 1) * P],
                                in_=gt[:, t * P:(t + 1) * P],
                                identity=ident_bf[:])
                        nc.vector.tensor_copy(
                            out=g_sb[:, t4 * 4 * P:t4 * 4 * P + tw * P],
                            in_=g_ps[:, :tw * P])

                    acc_ps = apps.tile([P, D + H], FP32, tag="accps")
                    att = pool.tile([P, TB * H], FP32, tag="att")

                    for g8 in range(n8):
                        t0 = g8 * 8
                        tw = min(8, TB - t0)
                        qe_ps = pps.tile([P, 8 * D], FP32, tag="qeps")
                        for j in range(tw):
                            t = t0 + j
                            nc.tensor.matmul(
                                out=qe_ps[:, j * D:(j + 1) * D],
                                lhsT=gt[:, t * P:(t + 1) * P],
                                rhs=qb_hi[:], start=True, stop=False)
                            nc.tensor.matmul(
                                out=qe_ps[:, j * D:(j + 1) * D],
                                lhsT=gt[:, t * P:(t + 1) * P],
                                rhs=qb_lo[:], start=False, stop=True)
                        # s = q_e * k ; att = head-sum(s)
                        s_sb = pool.tile([P, 8 * D], FP32, tag="ssb")
                        kv3 = kvg[:].rearrange("p (t c) -> p t c", c=2 * D)
                        nc.vector.tensor_tensor(
                            out=s_sb[:, :tw * D],
                            in0=qe_ps[:, :tw * D],
                            in1=kv3[:, t0:t0 + tw, 0:D],
                            op=mybir.AluOpType.mult)
                        nc.vector.tensor_reduce(
                            out=att[:, t0 * H:t0 * H + tw * H],
                            in_=s_sb[:, :tw * D].rearrange(
                                "p (g d) -> p g d", d=DH),
                            axis=mybir.AxisListType.X,
                            op=mybir.AluOpType.add)

                    # clip +-10, exponentiate
                    nc.vector.tensor_scalar_min(
                        out=att[:], in0=att[:], scalar1=10.0)
                    nc.vector.tensor_scalar_max(
                        out=att[:], in0=att[:], scalar1=-10.0)
                    w_sb = pool.tile([P, TB * H], FP32, tag="wsb")
                    nc.scalar.activation(
                        out=w_sb[:], in_=att[:],
                        func=mybir.ActivationFunctionType.Exp)

                    wv = pool.tile([P, TB * (D + H)], BF16, tag="wv")
                    wv3 = wv[:].rearrange("p (t c) -> p t c", c=D + H)
                    for g8 in range(n8):
                        t0 = g8 * 8
                        tw = min(8, TB - t0)
                        kv3 = kvg[:].rearrange("p (t c) -> p t c", c=2 * D)
                        w4 = w_sb[:, t0 * H:t0 * H + tw * H].rearrange(
                            "p (t h) -> p t h", h=H)
                        nc.vector.tensor_tensor(
                            out=wv3[:, t0:t0 + tw, :D].rearrange(
                                "p t (h f) -> p t h f", h=H),
                            in0=kv3[:, t0:t0 + tw, D:2 * D].rearrange(
                                "p t (h f) -> p t h f", h=H),
                            in1=w4.unsqueeze(3).to_broadcast((P, tw, H, DH)),
                            op=mybir.AluOpType.mult)
                    nc.vector.tensor_copy(
                        out=wv3[:, :, D:],
                        in_=w_sb[:].rearrange("p (t h) -> p t h", h=H))

                    for t in range(TB):
                        nc.tensor.matmul(
                            out=acc_ps[:],
                            lhsT=g_sb[:, t * P:(t + 1) * P],
                            rhs=wv[:, t * (D + H):(t + 1) * (D + H)],
                            start=(t == 0), stop=(t == TB - 1))

                    if debug_dump and b == 0:
                        nc.sync.dma_start(out=dbg["d_kvg"][:, :], in_=kvg[:])
                        nc.sync.dma_start(out=dbg["d_gt"][:, :], in_=gt[:])
                        nc.sync.dma_start(out=dbg["d_g"][:, :], in_=g_sb[:])
                        nc.sync.dma_start(out=dbg["d_w"][:, :], in_=w_sb[:])

                    # normalize: out = acc / (norm + 1e-8)
                    rec = pool.tile([P, H], FP32, tag="rec")
                    nc.vector.tensor_scalar_add(
                        out=rec[:], in0=acc_ps[:, D:], scalar1=1e-8)
                    nc.vector.reciprocal(out=rec[:], in_=rec[:])
                    outf = pool.tile([P, D], FP32, tag="outf")
                    nc.vector.tensor_tensor(
                        out=outf[:].rearrange("p (h f) -> p h f", h=H),
                        in0=acc_ps[:, :D].rearrange("p (h f) -> p h f", h=H),
                        in1=rec[:].unsqueeze(2).to_broadcast((P, H, DH)),
                        op=mybir.AluOpType.mult)
                    nc.sync.dma_start(
                        out=out_d[b * P:(b + 1) * P, :], in_=outf[:])

    nc.compile()
    return nc


# ----------------------------------------------------------------- interface
def kernel(all_embeddings, Wq, Wk, Wv, edge_index):
    global LAST_RESULT
    emb = np.ascontiguousarray(np.asarray(all_embeddings, dtype=np.float32))
    Wq = np.asarray(Wq, dtype=np.float32)
    Wk = np.asarray(Wk, dtype=np.float32)
    Wv = np.asarray(Wv, dtype=np.float32)

    cols_g, lrow2, TB = _preprocess(np.asarray(edge_index))

    emb_pad = np.zeros((N_PAD, D), dtype=np.float32)
    emb_pad[:N_NODES] = emb
    emb_t = np.ascontiguousarray(emb_pad.T)           # [D, N_PAD]
    w_kv = np.ascontiguousarray(np.concatenate([Wk, Wv], axis=1))

    nc = _build_program(TB)

    in_maps = []
    for c in range(NCORES):
        in_maps.append({
            "emb_t": emb_t,
            "emb_own_t": np.ascontiguousarray(
                emb_t[:, c * NPC:(c + 1) * NPC]),
            "w_kv": w_kv,
            "w_q": np.ascontiguousarray(Wq),
            "cols_g": np.ascontiguousarray(cols_g[c]),
            "lrow2": np.ascontiguousarray(lrow2[c]),
        })

    trace = bool(int(os.environ.get("GT_TRACE", "0")))
    res = bass_utils.run_bass_kernel_spmd(
        nc, in_maps, core_ids=list(range(NCORES)), trace=trace)
    LAST_RESULT = res

    out = np.empty((N_NODES, D), dtype=np.float32)
    for c in range(NCORES):
        lo = c * NPC
        hi = min((c + 1) * NPC, N_NODES)
        out[lo:hi] = res.results[c]["out"][:hi - lo]
    return out
